# revision 29
# baseline (speedup 1.0000x reference)
"""DeepSeek-style MLA decode attention (batch=8, 128 heads, cache 512) on 8 NeuronCores.

Sharding: tensor-parallel over heads (16 heads/core), bf16 on-device (fp16 o_proj).
 - q LoRA down-proj sharded over rank cols (exact per-core cq); Wkv_down sharded
   over input rows (partial c_kv). One small AllGather ships transposed
   cq/ckv-partials to every core early; each core then computes q for its own
   heads with a column shard of Wq_up (no big mid-kernel collective).
 - Phase A uses a masked-q layout: qTm block hb is [128,32] with only column
   hb%32 live, so the 8 score MMs of a super accumulate into one [32,512] PSUM
   tile whose rows are the real score rows -- no per-row extraction; softmax
   EXP reads the PSUM group tile directly.
 - k_cache host-pretransposed to [super, d, keys]; v_cache to [super, k, (c,t,d)].
 - o_proj input rows sharded by head; partial outputs ReduceScattered over the
   batch dim in 3 column chunks (overlapped with o_proj compute); core b
   returns batch b's final row.

Note: the reference's "new token" softmax is over a length-1 axis (== 1.0), so
k_new/Wk_up are dead and the new-token contribution is simply + v_new.
"""

import numpy as np
import ml_dtypes

import concourse.bass as bass
import concourse.mybir as mybir
import concourse.tile as tile
from concourse import bacc
from concourse import bass_utils
from concourse.masks import make_identity

NC_ = 8                      # cores
B = 8                        # batch
H = 128                      # total heads
HP = H // NC_                # 16 heads per core
D = 128                      # head dim
L = 512                      # cache len
HID = 7168
QL = 1536
QLP = QL // NC_              # 192
KVL = 512
KVRP = HID // NC_            # 896 input rows of Wkv_down per core
NH = HP * D                  # 2048 per-core head cols
SCALE = 1.0 / float(np.sqrt(D))
F32 = mybir.dt.float32
F16 = mybir.dt.float16
BF16 = mybir.dt.bfloat16
U8 = mybir.dt.uint8
BF = ml_dtypes.bfloat16

# o_proj column rounds: (start, end) in 512-col chunks of HID
ROUNDS = ((0, 6), (6, 12), (12, 14))


def build_nc():
    nc = bacc.Bacc(
        "TRN2",
        target_bir_lowering=False,
        debug=False,
        enable_asserts=False,
        num_devices=NC_,
    )
    xt = nc.dram_tensor("xt", [128, 56 * B], BF16, kind="ExternalInput").ap()
    xkv = nc.dram_tensor("xkv", [128, 7 * B], BF16, kind="ExternalInput").ap()
    wqd = nc.dram_tensor("wqd", [2, 128, 28 * QLP], BF16, kind="ExternalInput").ap()
    wkvd = nc.dram_tensor("wkvd", [128, 7 * KVL], BF16, kind="ExternalInput").ap()
    wq = nc.dram_tensor("wq", [4, 128, 16 * 512], BF16, kind="ExternalInput").ap()
    wvup = nc.dram_tensor("wvup", [2, 128, 2 * NH], BF16, kind="ExternalInput").ap()
    kt = nc.dram_tensor("kt", [16, 128, 4096], BF16, kind="ExternalInput").ap()
    v = nc.dram_tensor("v", [16, 128, 4096], BF16, kind="ExternalInput").ap()
    wos = [
        nc.dram_tensor(
            f"wo{r}", [8, 128, 2 * (n1 - n0) * 512], F16, kind="ExternalInput"
        ).ap()
        for r, (n0, n1) in enumerate(ROUNDS)
    ]
    o = nc.dram_tensor("o", [1, HID], F32, kind="ExternalOutput").ap()

    rg = [list(range(NC_))]

    with tile.TileContext(nc) as tc:
        with (
            tc.tile_pool(name="const", bufs=1) as constp,
            tc.tile_pool(name="sbuf", bufs=1) as sb,
            tc.tile_pool(name="stage", bufs=1) as stg,
            tc.tile_pool(name="wqdp", bufs=2) as wqdp,
            tc.tile_pool(name="wqp", bufs=2) as wqp,
            tc.tile_pool(name="ktp", bufs=6) as ktp,
            tc.tile_pool(name="vp", bufs=3) as vp,
            tc.tile_pool(name="psbank", bufs=6, space="PSUM") as psbank,
            tc.tile_pool(name="pstr", bufs=2, space="PSUM") as pstr,
            tc.tile_pool(name="dram", bufs=1, space="DRAM") as dram,
        ):
            ident = constp.tile([128, 128], F32)
            make_identity(nc, ident[:])
            id8 = ident[0:8, 0:8]
            # uint8 one-hot columns for CopyPredicated masks (must be int dtype)
            identu8 = constp.tile([128, 128], U8, tag="identu8")
            nc.vector.tensor_copy(identu8[:], ident[:])

            # ---------------- LoRA down: cq = x @ Wq_down_c, ckv partial ------------
            xt_sb = constp.tile([128, 56 * B], BF16, tag="xt")
            nc.sync.dma_start(out=xt_sb[:], in_=xt)
            xkv_sb = constp.tile([128, 7 * B], BF16, tag="xkv")
            nc.sync.dma_start(out=xkv_sb[:], in_=xkv)
            wkvd_sb = constp.tile([128, 7 * KVL], BF16, tag="wkvd")
            nc.sync.dma_start(out=wkvd_sb[:], in_=wkvd)

            ps_cq = psbank.tile([8, 512], F32, tag="bank")
            ps_ckv = psbank.tile([8, 512], F32, tag="bank")
            wqd_ts = []
            for j in range(2):
                wqd_t = wqdp.tile([128, 28 * QLP], BF16, tag="wqd")
                nc.sync.dma_start(out=wqd_t[:], in_=wqd[j])
                wqd_ts.append(wqd_t)
            for i in range(56):
                j, ii = divmod(i, 28)
                nc.tensor.matmul(
                    ps_cq[:8, 0:QLP],
                    xt_sb[:, i * B:(i + 1) * B],
                    wqd_ts[j][:, ii * QLP:(ii + 1) * QLP],
                    start=(i == 0), stop=(i == 55),
                )
            for i in range(7):
                nc.tensor.matmul(
                    ps_ckv[:8, :],
                    xkv_sb[:, i * B:(i + 1) * B],
                    wkvd_sb[:, i * KVL:(i + 1) * KVL],
                    start=(i == 0), stop=(i == 6),
                )
            cdq = sb.tile([8, QLP], F32, tag="cdq")
            nc.vector.tensor_copy(cdq[:], ps_cq[:8, 0:QLP])
            cdkv = sb.tile([8, KVL], F32, tag="cdkv")
            nc.vector.tensor_copy(cdkv[:], ps_ckv[:8, :])

            # transposes -> ag_in staging [128, 6*8] f32
            # x-cols: 0: cq rows 0-127, 1: cq rows 128-191 (parts 0-63, rest zero),
            #         2-5: ckv chunks of 128
            ps_cqT = pstr.tile([128, 16], F32, tag="tr")
            nc.tensor.transpose(ps_cqT[0:128, 0:8], cdq[:, 0:128], id8)
            nc.tensor.transpose(ps_cqT[0:64, 8:16], cdq[:, 128:192], id8)
            ps_ckvT = pstr.tile([128, 32], F32, tag="tr")
            for j in range(4):
                nc.tensor.transpose(
                    ps_ckvT[0:128, j * 8:(j + 1) * 8],
                    cdkv[:, j * 128:(j + 1) * 128],
                    id8,
                )
            ag_in_sb = sb.tile([128, 48], F32, tag="agin")
            nc.vector.memset(ag_in_sb[:, 8:16], 0.0)
            nc.vector.tensor_copy(ag_in_sb[:, 0:8], ps_cqT[:, 0:8])
            nc.vector.tensor_copy(ag_in_sb[0:64, 8:16], ps_cqT[0:64, 8:16])
            nc.vector.tensor_copy(ag_in_sb[:, 16:48], ps_ckvT[:, 0:32])

            ag_in = dram.tile([128, 48], F32, tag="agi")
            nc.sync.dma_start(out=ag_in[:], in_=ag_in_sb[:])
            ag_out = dram.tile([NC_ * 128, 48], F32, tag="ago")
            nc.gpsimd.collective_compute(
                "AllGather",
                mybir.AluOpType.bypass,
                replica_groups=rg,
                ins=[ag_in.opt()],
                outs=[ag_out.opt()],
            )
            # cq_stage [128, (r, x, b)] f32: one clean DMA (1.5KB/partition runs)
            cq_stage = sb.tile([128, 8 * 48], F32, tag="cqstage")
            nc.sync.dma_start(
                out=cq_stage[:].rearrange("p (r c) -> p r c", r=8),
                in_=ag_out[:].rearrange("(r p) c -> p r c", p=128),
            )
            cqmm = sb.tile([128, 8 * 48], BF16, tag="cqmm")
            nc.vector.tensor_copy(cqmm[:], cq_stage[:])

            # ckv full = sum of the 8 gathered partials -> ckvT16 [128, 4*8] bf16
            ckvT = sb.tile([128, 32], F32, tag="ckvT")
            nc.vector.tensor_copy(ckvT[:], cq_stage[:, 16:48])
            for r in range(1, 8):
                base = r * 48 + 16
                nc.vector.tensor_add(ckvT[:], ckvT[:], cq_stage[:, base:base + 32])
            ckvT16 = sb.tile([128, 32], BF16, tag="ckvT16")
            nc.vector.tensor_copy(ckvT16[:], ckvT[:])

            # wvup DMAs through the (now idle) wqd pool
            wvup_ts = []
            for j in range(2):
                wv_t = wqdp.tile([128, 2 * NH], BF16, tag="wqd", name=f"wvup{j}")
                nc.sync.dma_start(out=wv_t[:], in_=wvup[j])
                wvup_ts.append(wv_t)
            vnew = sb.tile([8, NH], F32, tag="vnew")
            for n in range(4):
                ps_v = psbank.tile([8, 512], F32, tag="bank")
                for cc in range(4):
                    nc.tensor.matmul(
                        ps_v[:8, :],
                        ckvT16[:, cc * 8:(cc + 1) * 8],
                        wvup_ts[cc // 2][:, (cc % 2) * NH + n * 512:
                                         (cc % 2) * NH + (n + 1) * 512],
                        start=(cc == 0), stop=(cc == 3),
                    )
                nc.vector.tensor_copy(vnew[:, n * 512:(n + 1) * 512], ps_v[:8, :])
            ps_vT = pstr.tile([128, 128], F32, tag="tr")
            for h in range(HP):
                nc.tensor.transpose(
                    ps_vT[0:128, h * 8:(h + 1) * 8],
                    vnew[:, h * D:(h + 1) * D],
                    id8,
                )
            vnewT = sb.tile([128, 128], F32, tag="vnewT")
            nc.vector.tensor_copy(vnewT[:], ps_vT[:])

            qstage = sb.tile([8, NH], F32, tag="qstage")
            qT = sb.tile([128, 128], BF16, tag="qT")
            # masked q: qTm block hb = [128, 32], only column hb%32 live
            qTm = sb.tile([128, 128 * 32], BF16, tag="qTm")
            nc.vector.memset(qTm[:], 0.0)

            # ---------------- phases A+B: per-group pipelined attention ----------
            # group a = hb 32a..32a+32. Phase A accumulates the group's 32 score
            # rows into one base-0 [32,512] PSUM tile via the masked-q layout;
            # per-group softmax; phase B for the group uses [32]-wide probsT
            # column slices so it starts as soon as the group's softmax is done.
            probsT = sb.tile([128, 512], BF16, tag="probsT")
            attnT = sb.tile([128, 128], F16, tag="attnT")
            id32 = ident[0:32, 0:32]
            for a in range(4):
                pa = 32 * a
                # --- q_up chunk a: 16 MMs ping-ponged across two banks ---
                wq_t = wqp.tile([128, 16 * 512], BF16, tag="wq")
                nc.sync.dma_start(out=wq_t[:], in_=wq[a])
                ps_qa = psbank.tile([8, 512], F32, tag="bank", name=f"ps_qa{a}")
                ps_qb = psbank.tile([8, 512], F32, tag="bank", name=f"ps_qb{a}")
                for r in range(8):
                    nc.tensor.matmul(
                        ps_qa[:8, :],
                        cqmm[0:128, r * 48:r * 48 + 8],
                        wq_t[0:128, (2 * r) * 512:(2 * r + 1) * 512],
                        start=(r == 0), stop=(r == 7),
                    )
                    nc.tensor.matmul(
                        ps_qb[:8, :],
                        cqmm[0:64, r * 48 + 8:r * 48 + 16],
                        wq_t[0:64, (2 * r + 1) * 512:(2 * r + 2) * 512],
                        start=(r == 0), stop=(r == 7),
                    )
                qsl = qstage[:, a * 512:(a + 1) * 512]
                nc.vector.tensor_copy(qsl, ps_qa[:8, :])
                nc.vector.tensor_add(qsl, qsl, ps_qb[:8, :])
                ps_qT = pstr.tile([128, 32], F32, tag="tr")
                for hh in range(4):
                    h = 4 * a + hh
                    nc.tensor.transpose(
                        ps_qT[0:128, hh * 8:(hh + 1) * 8],
                        qstage[:, h * D:(h + 1) * D],
                        id8,
                    )
                nc.vector.tensor_copy(qT[:, pa:pa + 32], ps_qT[:])

                ps_g = psbank.tile([32, 512], F32, tag="bank", name=f"ps_g{a}")
                for s in range(4 * a, 4 * a + 4):
                    kt_t = ktp.tile([128, 4096], BF16, tag="kt")
                    nc.sync.dma_start(out=kt_t[:], in_=kt[s])
                    for u in range(8):
                        hb = 8 * s + u
                        nc.vector.tensor_copy(
                            qTm[:, hb * 32 + (hb % 32):hb * 32 + (hb % 32) + 1],
                            qT[:, hb:hb + 1],
                        )
                        nc.tensor.matmul(
                            ps_g[0:32, :],
                            qTm[:, hb * 32:(hb + 1) * 32],
                            kt_t[:, u * 512:(u + 1) * 512],
                            start=(s % 4 == 0 and u == 0),
                            stop=(s % 4 == 3 and u == 7),
                        )
                probs_a = sb.tile([32, 512], F32, tag=f"probs{a}")
                denom_a = sb.tile([32, 1], F32, tag=f"denom{a}")
                nc.scalar.activation(
                    probs_a[:], ps_g[0:32, :],
                    mybir.ActivationFunctionType.Exp,
                    scale=SCALE, accum_out=denom_a[:],
                )
                recip_a = sb.tile([32, 1], F32, tag=f"recip{a}")
                nc.vector.reciprocal(recip_a[:], denom_a[:])
                probsn_a = sb.tile([32, 512], F32, tag=f"probsn{a}")
                nc.vector.tensor_scalar_mul(probsn_a[:], probs_a[:], recip_a[:])
                for cc in range(4):
                    ps_pT = pstr.tile([128, 32], F32, tag="tr")
                    nc.tensor.transpose(
                        ps_pT[:], probsn_a[0:32, cc * 128:(cc + 1) * 128], id32
                    )
                    nc.vector.tensor_copy(
                        probsT[:, cc * 128 + pa:cc * 128 + pa + 32], ps_pT[:]
                    )

                # phase B for this group
                attn_a = sb.tile([32, 128], F32, tag=f"attn{a}")
                for s in range(4 * a, 4 * a + 4):
                    v_t = vp.tile([128, 4096], BF16, tag="v")
                    nc.sync.dma_start(out=v_t[:], in_=v[s])
                    for gg in range(2):
                        g = 2 * s + gg
                        ps_a = psbank.tile(
                            [32, 512], F32, tag="bank", name=f"ps_b{g}"
                        )
                        for cc in range(4):
                            nc.tensor.matmul(
                                ps_a[0:32, :],
                                probsT[:, cc * 128 + pa:cc * 128 + pa + 32],
                                v_t[:, gg * 2048 + cc * 512:
                                    gg * 2048 + (cc + 1) * 512],
                                start=(cc == 0), stop=(cc == 3),
                            )
                        for u in range(4):
                            hb = 4 * g + u
                            j = hb % 32
                            nc.vector.copy_predicated(
                                attn_a[0:32, :],
                                identu8[0:32, j:j + 1].broadcast_to((32, 128)),
                                ps_a[0:32, u * 128:(u + 1) * 128],
                            )
                # attnT block = attn_a^T + vnewT block (f16 for o_proj)
                ps_aT = pstr.tile([128, 32], F32, tag="tr")
                nc.tensor.transpose(ps_aT[:], attn_a[:], id32)
                nc.vector.tensor_add(
                    attnT[:, pa:pa + 32], ps_aT[:], vnewT[:, pa:pa + 32]
                )

            # ---------------- phase C: o_part = attn^T @ Wo_c, chunked RS ----------
            o_rss = []
            for r, (n0, n1) in enumerate(ROUNDS):
                nn = n1 - n0
                ps_os = [
                    psbank.tile([8, 512], F32, tag="bank", name=f"ps_o{r}_{i}")
                    for i in range(nn)
                ]
                for hp in range(8):
                    wo_t = vp.tile([128, 2 * 6 * 512], F16, tag="v")
                    nc.sync.dma_start(out=wo_t[:, 0:2 * nn * 512], in_=wos[r][hp])
                    for i2 in range(2):
                        h = 2 * hp + i2
                        for i in range(nn):
                            nc.tensor.matmul(
                                ps_os[i][:8, :],
                                attnT[:, h * 8:(h + 1) * 8],
                                wo_t[:, (i2 * nn + i) * 512:(i2 * nn + i + 1) * 512],
                                start=(h == 0), stop=(h == 15),
                            )
                ostage = stg.tile([8, 6 * 512], F32, tag="ostage")
                for i in range(nn):
                    nc.vector.tensor_copy(
                        ostage[:, i * 512:(i + 1) * 512], ps_os[i][:8, :]
                    )
                o_bounce = dram.tile([B, nn * 512], F32, tag=f"ob{r}")
                nc.sync.dma_start(out=o_bounce[:], in_=ostage[:, 0:nn * 512])
                o_rs = dram.tile([1, nn * 512], F32, tag=f"ors{r}")
                nc.gpsimd.collective_compute(
                    "ReduceScatter",
                    mybir.AluOpType.add,
                    replica_groups=rg,
                    ins=[o_bounce.opt()],
                    outs=[o_rs.opt()],
                )
                o_rss.append((o_rs, n0, n1))

            for o_rs, n0, n1 in o_rss:
                nc.sync.dma_start(out=o[0:1, n0 * 512:n1 * 512], in_=o_rs[:])

    nc.compile()
    return nc


_NC_CACHE = None


def _get_nc():
    global _NC_CACHE
    if _NC_CACHE is None:
        _NC_CACHE = build_nc()
    return _NC_CACHE


def make_in_maps(x, k_cache, v_cache, Wq_down, Wq_up, Wkv_down, Wv_up, Wo):
    f16 = np.float16
    x2 = np.asarray(x, dtype=np.float32).reshape(B, HID).T  # [7168, 8]
    xt_t = np.ascontiguousarray(
        x2.reshape(56, 128, B).transpose(1, 0, 2).reshape(128, 56 * B).astype(BF)
    )
    Wq_down = np.asarray(Wq_down, dtype=np.float32)
    Wq_up = np.asarray(Wq_up, dtype=np.float32)
    Wkv_down = np.asarray(Wkv_down, dtype=np.float32)
    Wv_up = np.asarray(Wv_up, dtype=np.float32)
    Wo = np.asarray(Wo, dtype=np.float32)
    k_cache = np.asarray(k_cache, dtype=np.float32)
    v_cache = np.asarray(v_cache, dtype=np.float32)

    in_maps = []
    for c in range(NC_):
        hs = slice(c * HP, (c + 1) * HP)
        wqd_c = (
            Wq_down[:, c * QLP:(c + 1) * QLP]
            .reshape(2, 28, 128, QLP).transpose(0, 2, 1, 3)
            .reshape(2, 128, 28 * QLP).astype(BF)
        )
        wkvd_c = (
            Wkv_down[c * KVRP:(c + 1) * KVRP, :]
            .reshape(7, 128, KVL).transpose(1, 0, 2).reshape(128, 7 * KVL)
            .astype(BF)
        )
        xkv_c = np.ascontiguousarray(
            x2.reshape(56, 128, B)[7 * c:7 * c + 7]
            .transpose(1, 0, 2).reshape(128, 7 * B).astype(BF)
        )
        wq_shard = Wq_up[:, c * NH:(c + 1) * NH]
        # pad rows to 256 per rank (rows r*256+192..255 zero) so the K=64
        # chunk sits at partitions 0..64 of its own x-column
        wq_pad = np.zeros((2048, NH), np.float32)
        for r in range(8):
            wq_pad[r * 256:r * 256 + QLP] = wq_shard[r * QLP:(r + 1) * QLP]
        wq_c = np.stack([
            wq_pad[:, n * 512:(n + 1) * 512]
            .reshape(16, 128, 512).transpose(1, 0, 2).reshape(128, 16 * 512)
            for n in range(4)
        ]).astype(BF)
        wvup_c = (
            Wv_up[:, c * NH:(c + 1) * NH]
            .reshape(2, 2, 128, NH).transpose(0, 2, 1, 3).reshape(2, 128, 2 * NH)
            .astype(BF)
        )
        kt_c = (
            k_cache[:, hs]
            .transpose(1, 0, 3, 2)          # (16, 8, 128, 512) [h, b, d, k]
            .reshape(32, 4, 128, 512)       # [g, t, d, k]
            .transpose(0, 2, 1, 3)          # [g, d, t, k]
            .reshape(16, 2, 128, 2048)      # [s, g2, d, tk]
            .transpose(0, 2, 1, 3)
            .reshape(16, 128, 4096)
            .astype(BF)
        )
        v_c = (
            v_cache[:, hs]
            .transpose(1, 0, 2, 3)          # (16, 8, 512, 128) [h, b, l, d]
            .reshape(32, 4, 4, 128, 128)    # [g, t, c, k, d]
            .transpose(0, 3, 2, 1, 4)       # [g, k, c, t, d]
            .reshape(16, 2, 128, 2048)
            .transpose(0, 2, 1, 3)
            .reshape(16, 128, 4096)
            .astype(BF)
        )
        wo_shard = Wo[c * NH:(c + 1) * NH, :]  # [2048, 7168]
        wo_cs = []
        for (n0, n1) in ROUNDS:
            nn = n1 - n0
            wo_r = (
                wo_shard[:, n0 * 512:n1 * 512]
                .reshape(8, 2, 128, nn * 512).transpose(0, 2, 1, 3)
                .reshape(8, 128, 2 * nn * 512).astype(f16)
            )
            wo_cs.append(wo_r)
        in_maps.append(
            {
                "xt": xt_t,
                "xkv": xkv_c,
                "wqd": np.ascontiguousarray(wqd_c),
                "wkvd": np.ascontiguousarray(wkvd_c),
                "wq": np.ascontiguousarray(wq_c),
                "wvup": np.ascontiguousarray(wvup_c),
                "kt": np.ascontiguousarray(kt_c),
                "v": np.ascontiguousarray(v_c),
                "wo0": np.ascontiguousarray(wo_cs[0]),
                "wo1": np.ascontiguousarray(wo_cs[1]),
                "wo2": np.ascontiguousarray(wo_cs[2]),
            }
        )
    return in_maps


def kernel(x, k_cache, v_cache, Wq_down, Wq_up, Wkv_down, Wk_up, Wv_up, Wo, **_):
    in_maps = make_in_maps(
        np.asarray(x), np.asarray(k_cache), np.asarray(v_cache),
        np.asarray(Wq_down), np.asarray(Wq_up),
        np.asarray(Wkv_down), np.asarray(Wv_up), np.asarray(Wo),
    )
    nc = _get_nc()
    res = bass_utils.run_bass_kernel_spmd(nc, in_maps, core_ids=list(range(NC_)))
    out = np.stack([res.results[b]["o"] for b in range(B)], axis=0)  # (8, 1, 7168)
    return np.ascontiguousarray(out, dtype=np.float32)


# revision 31
# speedup vs baseline: 1.0032x; 1.0032x over previous
"""DeepSeek-style MLA decode attention (batch=8, 128 heads, cache 512) on 8 NeuronCores.

Sharding: tensor-parallel over heads (16 heads/core), bf16 on-device (fp16 o_proj).
 - q LoRA down-proj sharded over rank cols (exact per-core cq); Wkv_down sharded
   over input rows (partial c_kv). One small AllGather ships transposed
   cq/ckv-partials to every core early; each core then computes q for its own
   heads with a column shard of Wq_up (no big mid-kernel collective).
 - Phase A uses a masked-q layout: qTm block hb is [128,32] with only column
   hb%32 live, so the 8 score MMs of a super accumulate into one [32,512] PSUM
   tile whose rows are the real score rows -- no per-row extraction; softmax
   EXP reads the PSUM group tile directly.
 - k_cache host-pretransposed to [super, d, keys]; v_cache to [super, k, (c,t,d)].
 - o_proj input rows sharded by head; partial outputs ReduceScattered over the
   batch dim in 3 column chunks (overlapped with o_proj compute); core b
   returns batch b's final row.

Note: the reference's "new token" softmax is over a length-1 axis (== 1.0), so
k_new/Wk_up are dead and the new-token contribution is simply + v_new.
"""

import numpy as np
import ml_dtypes

import concourse.bass as bass
import concourse.mybir as mybir
import concourse.tile as tile
from concourse import bacc
from concourse import bass_utils
from concourse.masks import make_identity

NC_ = 8                      # cores
B = 8                        # batch
H = 128                      # total heads
HP = H // NC_                # 16 heads per core
D = 128                      # head dim
L = 512                      # cache len
HID = 7168
QL = 1536
QLP = QL // NC_              # 192
KVL = 512
KVRP = HID // NC_            # 896 input rows of Wkv_down per core
NH = HP * D                  # 2048 per-core head cols
SCALE = 1.0 / float(np.sqrt(D))
F32 = mybir.dt.float32
F16 = mybir.dt.float16
BF16 = mybir.dt.bfloat16
U8 = mybir.dt.uint8
BF = ml_dtypes.bfloat16

# o_proj column rounds: (start, end) in 512-col chunks of HID
ROUNDS = ((0, 6), (6, 12), (12, 14))


def build_nc():
    nc = bacc.Bacc(
        "TRN2",
        target_bir_lowering=False,
        debug=False,
        enable_asserts=False,
        num_devices=NC_,
    )
    xt = nc.dram_tensor("xt", [128, 56 * B], BF16, kind="ExternalInput").ap()
    xkv = nc.dram_tensor("xkv", [128, 7 * B], BF16, kind="ExternalInput").ap()
    wqd = nc.dram_tensor("wqd", [2, 128, 28 * QLP], BF16, kind="ExternalInput").ap()
    wkvd = nc.dram_tensor("wkvd", [128, 7 * KVL], BF16, kind="ExternalInput").ap()
    wq = nc.dram_tensor("wq", [4, 128, 16 * 512], BF16, kind="ExternalInput").ap()
    wvup = nc.dram_tensor("wvup", [2, 128, 2 * NH], BF16, kind="ExternalInput").ap()
    kt = nc.dram_tensor("kt", [16, 128, 4096], BF16, kind="ExternalInput").ap()
    v = nc.dram_tensor("v", [16, 128, 4096], BF16, kind="ExternalInput").ap()
    wos = [
        nc.dram_tensor(
            f"wo{r}", [8, 128, 2 * (n1 - n0) * 512], F16, kind="ExternalInput"
        ).ap()
        for r, (n0, n1) in enumerate(ROUNDS)
    ]
    o = nc.dram_tensor("o", [1, HID], F32, kind="ExternalOutput").ap()

    rg = [list(range(NC_))]

    with tile.TileContext(nc) as tc:
        with (
            tc.tile_pool(name="const", bufs=1) as constp,
            tc.tile_pool(name="sbuf", bufs=1) as sb,
            tc.tile_pool(name="stage", bufs=1) as stg,
            tc.tile_pool(name="wqdp", bufs=1) as wqdp,
            tc.tile_pool(name="wqp", bufs=2) as wqp,
            tc.tile_pool(name="ktp", bufs=4) as ktp,
            tc.tile_pool(name="vp", bufs=3) as vp,
            tc.tile_pool(name="wop", bufs=2) as wop,
            tc.tile_pool(name="psbank", bufs=6, space="PSUM") as psbank,
            tc.tile_pool(name="pstr", bufs=2, space="PSUM") as pstr,
            tc.tile_pool(name="dram", bufs=1, space="DRAM") as dram,
        ):
            ident = constp.tile([128, 128], F32)
            make_identity(nc, ident[:])
            id8 = ident[0:8, 0:8]
            # uint8 one-hot columns for CopyPredicated masks (must be int dtype)
            identu8 = constp.tile([128, 128], U8, tag="identu8")
            nc.vector.tensor_copy(identu8[:], ident[:])

            # ---------------- LoRA down: cq = x @ Wq_down_c, ckv partial ------------
            xt_sb = constp.tile([128, 56 * B], BF16, tag="xt")
            nc.sync.dma_start(out=xt_sb[:], in_=xt)
            xkv_sb = constp.tile([128, 7 * B], BF16, tag="xkv")
            nc.sync.dma_start(out=xkv_sb[:], in_=xkv)
            wkvd_sb = constp.tile([128, 7 * KVL], BF16, tag="wkvd")
            nc.sync.dma_start(out=wkvd_sb[:], in_=wkvd)

            ps_cq = psbank.tile([8, 512], F32, tag="bank")
            ps_ckv = psbank.tile([8, 512], F32, tag="bank")
            wqd_ts = []
            for j in range(2):
                wqd_t = wqdp.tile([128, 28 * QLP], BF16, tag="wqd")
                nc.sync.dma_start(out=wqd_t[:], in_=wqd[j])
                wqd_ts.append(wqd_t)
            for i in range(56):
                j, ii = divmod(i, 28)
                nc.tensor.matmul(
                    ps_cq[:8, 0:QLP],
                    xt_sb[:, i * B:(i + 1) * B],
                    wqd_ts[j][:, ii * QLP:(ii + 1) * QLP],
                    start=(i == 0), stop=(i == 55),
                )
            for i in range(7):
                nc.tensor.matmul(
                    ps_ckv[:8, :],
                    xkv_sb[:, i * B:(i + 1) * B],
                    wkvd_sb[:, i * KVL:(i + 1) * KVL],
                    start=(i == 0), stop=(i == 6),
                )
            cdq = sb.tile([8, QLP], F32, tag="cdq")
            nc.vector.tensor_copy(cdq[:], ps_cq[:8, 0:QLP])
            cdkv = sb.tile([8, KVL], F32, tag="cdkv")
            nc.vector.tensor_copy(cdkv[:], ps_ckv[:8, :])

            # transposes -> ag_in staging [128, 6*8] f32
            # x-cols: 0: cq rows 0-127, 1: cq rows 128-191 (parts 0-63, rest zero),
            #         2-5: ckv chunks of 128
            ps_cqT = pstr.tile([128, 16], F32, tag="tr")
            nc.tensor.transpose(ps_cqT[0:128, 0:8], cdq[:, 0:128], id8)
            nc.tensor.transpose(ps_cqT[0:64, 8:16], cdq[:, 128:192], id8)
            ps_ckvT = pstr.tile([128, 32], F32, tag="tr")
            for j in range(4):
                nc.tensor.transpose(
                    ps_ckvT[0:128, j * 8:(j + 1) * 8],
                    cdkv[:, j * 128:(j + 1) * 128],
                    id8,
                )
            ag_in_sb = sb.tile([128, 48], F32, tag="agin")
            nc.vector.memset(ag_in_sb[:, 8:16], 0.0)
            nc.vector.tensor_copy(ag_in_sb[:, 0:8], ps_cqT[:, 0:8])
            nc.vector.tensor_copy(ag_in_sb[0:64, 8:16], ps_cqT[0:64, 8:16])
            nc.vector.tensor_copy(ag_in_sb[:, 16:48], ps_ckvT[:, 0:32])

            ag_in = dram.tile([128, 48], F32, tag="agi")
            nc.sync.dma_start(out=ag_in[:], in_=ag_in_sb[:])
            ag_out = dram.tile([NC_ * 128, 48], F32, tag="ago")
            nc.gpsimd.collective_compute(
                "AllGather",
                mybir.AluOpType.bypass,
                replica_groups=rg,
                ins=[ag_in.opt()],
                outs=[ag_out.opt()],
            )
            # cq_stage [128, (r, x, b)] f32: one clean DMA (1.5KB/partition runs)
            cq_stage = sb.tile([128, 8 * 48], F32, tag="cqstage")
            nc.sync.dma_start(
                out=cq_stage[:].rearrange("p (r c) -> p r c", r=8),
                in_=ag_out[:].rearrange("(r p) c -> p r c", p=128),
            )
            cqmm = sb.tile([128, 8 * 48], BF16, tag="cqmm")
            nc.vector.tensor_copy(cqmm[:], cq_stage[:])

            # PE warm-up: ~6us of dense dummy MMs anchored on cqmm so they run
            # right after the AllGather lands, flipping HAM to full clock
            # before the q_up chain
            ps_w0 = psbank.tile([8, 512], F32, tag="bank", name="ps_warm0")
            ps_w1 = psbank.tile([8, 512], F32, tag="bank", name="ps_warm1")
            for w in range(16):
                ps_w = ps_w0 if w % 2 == 0 else ps_w1
                nc.tensor.matmul(
                    ps_w[:8, 0:384],
                    cqmm[0:128, 0:8],
                    cqmm[0:128, 0:384],
                    start=(w < 2), stop=(w >= 14),
                )

            # ---------------- q_own = cq @ Wq_up_c  (8, 2048) ----------------
            # per rank r: K=128 chunk (cols r*48..) + K=64 chunk (cols r*48+8..)
            qstage = sb.tile([8, NH], F32, tag="qstage")
            for n in range(4):
                wq_t = wqp.tile([128, 16 * 512], BF16, tag="wq")
                nc.sync.dma_start(out=wq_t[:], in_=wq[n])
                # ping-pong two banks so fill/drain of same-bank accumulation
                # chains overlap; merge with copy+add
                ps_qa = psbank.tile([8, 512], F32, tag="bank", name=f"ps_qa{n}")
                ps_qb = psbank.tile([8, 512], F32, tag="bank", name=f"ps_qb{n}")
                for r in range(8):
                    tgt = ps_qa if r % 2 == 0 else ps_qb
                    nc.tensor.matmul(
                        tgt[:8, :],
                        cqmm[0:128, r * 48:r * 48 + 8],
                        wq_t[0:128, (2 * r) * 512:(2 * r + 1) * 512],
                        start=(r < 2), stop=False,
                    )
                    nc.tensor.matmul(
                        tgt[:8, :],
                        cqmm[0:64, r * 48 + 8:r * 48 + 16],
                        wq_t[0:64, (2 * r + 1) * 512:(2 * r + 2) * 512],
                        start=False, stop=(r >= 6),
                    )
                nc.vector.tensor_copy(qstage[:, n * 512:(n + 1) * 512], ps_qa[:8, :])
                nc.vector.tensor_add(
                    qstage[:, n * 512:(n + 1) * 512],
                    qstage[:, n * 512:(n + 1) * 512],
                    ps_qb[:8, :],
                )

            # ckv full = sum of the 8 gathered partials -> ckvT16 [128, 4*8] bf16
            ckvT = sb.tile([128, 32], F32, tag="ckvT")
            nc.vector.tensor_copy(ckvT[:], cq_stage[:, 16:48])
            for r in range(1, 8):
                base = r * 48 + 16
                nc.vector.tensor_add(ckvT[:], ckvT[:], cq_stage[:, base:base + 32])
            ckvT16 = sb.tile([128, 32], BF16, tag="ckvT16")
            nc.vector.tensor_copy(ckvT16[:], ckvT[:])

            # qT [128 d, 128 hb] bf16 via 16 transposes
            ps_qT = pstr.tile([128, 128], F32, tag="tr")
            for h in range(HP):
                nc.tensor.transpose(
                    ps_qT[0:128, h * 8:(h + 1) * 8],
                    qstage[:, h * D:(h + 1) * D],
                    id8,
                )
            qT = sb.tile([128, 128], BF16, tag="qT")
            for n in range(4):
                nc.vector.tensor_copy(
                    qT[:, 32 * n:32 * n + 32], ps_qT[:, 32 * n:32 * n + 32]
                )

            # masked q: qTm block hb = [128, 32], only column hb%32 live
            qTm = sb.tile([128, 128 * 32], BF16, tag="qTm")
            nc.vector.memset(qTm[:], 0.0)

            # ---------------- v_new = ckv @ Wv_up_c (8, 2048) ----------------
            wvup_ts = []
            for j in range(2):
                wv_t = wqp.tile([128, 2 * NH], BF16, tag="wq", name=f"wvup{j}")
                nc.sync.dma_start(out=wv_t[:], in_=wvup[j])
                wvup_ts.append(wv_t)
            vnew = sb.tile([8, NH], F32, tag="vnew")
            for n in range(4):
                ps_v = psbank.tile([8, 512], F32, tag="bank")
                for cc in range(4):
                    nc.tensor.matmul(
                        ps_v[:8, :],
                        ckvT16[:, cc * 8:(cc + 1) * 8],
                        wvup_ts[cc // 2][:, (cc % 2) * NH + n * 512:
                                         (cc % 2) * NH + (n + 1) * 512],
                        start=(cc == 0), stop=(cc == 3),
                    )
                nc.vector.tensor_copy(vnew[:, n * 512:(n + 1) * 512], ps_v[:8, :])
            ps_vT = pstr.tile([128, 128], F32, tag="tr")
            for h in range(HP):
                nc.tensor.transpose(
                    ps_vT[0:128, h * 8:(h + 1) * 8],
                    vnew[:, h * D:(h + 1) * D],
                    id8,
                )
            vnewT = sb.tile([128, 128], F32, tag="vnewT")
            nc.vector.tensor_copy(vnewT[:], ps_vT[:])

            # ---------------- phases A+B: per-group pipelined attention ----------
            # group a = hb 32a..32a+32. Phase A accumulates the group's 32 score
            # rows into one base-0 [32,512] PSUM tile via the masked-q layout;
            # per-group softmax; phase B for the group uses [32]-wide probsT
            # column slices so it starts as soon as the group's softmax is done.
            probsT = sb.tile([128, 512], BF16, tag="probsT")
            attnT = sb.tile([128, 128], F16, tag="attnT")
            id32 = ident[0:32, 0:32]
            for a in range(4):
                pa = 32 * a
                ps_g = psbank.tile([32, 512], F32, tag="bank", name=f"ps_g{a}")
                for s in range(4 * a, 4 * a + 4):
                    kt_t = ktp.tile([128, 4096], BF16, tag="kt")
                    nc.sync.dma_start(out=kt_t[:], in_=kt[s])
                    for u in range(8):
                        hb = 8 * s + u
                        nc.vector.tensor_copy(
                            qTm[:, hb * 32 + (hb % 32):hb * 32 + (hb % 32) + 1],
                            qT[:, hb:hb + 1],
                        )
                        nc.tensor.matmul(
                            ps_g[0:32, :],
                            qTm[:, hb * 32:(hb + 1) * 32],
                            kt_t[:, u * 512:(u + 1) * 512],
                            start=(s % 4 == 0 and u == 0),
                            stop=(s % 4 == 3 and u == 7),
                        )
                probs_a = sb.tile([32, 512], F32, tag=f"probs{a}")
                denom_a = sb.tile([32, 1], F32, tag=f"denom{a}")
                nc.scalar.activation(
                    probs_a[:], ps_g[0:32, :],
                    mybir.ActivationFunctionType.Exp,
                    scale=SCALE, accum_out=denom_a[:],
                )
                recip_a = sb.tile([32, 1], F32, tag=f"recip{a}")
                nc.vector.reciprocal(recip_a[:], denom_a[:])
                probsn_a = sb.tile([32, 512], F32, tag=f"probsn{a}")
                nc.vector.tensor_scalar_mul(probsn_a[:], probs_a[:], recip_a[:])
                for cc in range(4):
                    ps_pT = pstr.tile([128, 32], F32, tag="tr")
                    nc.tensor.transpose(
                        ps_pT[:], probsn_a[0:32, cc * 128:(cc + 1) * 128], id32
                    )
                    nc.vector.tensor_copy(
                        probsT[:, cc * 128 + pa:cc * 128 + pa + 32], ps_pT[:]
                    )

                # phase B for this group
                attn_a = sb.tile([32, 128], F32, tag=f"attn{a}")
                for s in range(4 * a, 4 * a + 4):
                    v_t = vp.tile([128, 4096], BF16, tag="v")
                    nc.sync.dma_start(out=v_t[:], in_=v[s])
                    for gg in range(2):
                        g = 2 * s + gg
                        ps_a = psbank.tile(
                            [32, 512], F32, tag="bank", name=f"ps_b{g}"
                        )
                        for cc in range(4):
                            nc.tensor.matmul(
                                ps_a[0:32, :],
                                probsT[:, cc * 128 + pa:cc * 128 + pa + 32],
                                v_t[:, gg * 2048 + cc * 512:
                                    gg * 2048 + (cc + 1) * 512],
                                start=(cc == 0), stop=(cc == 3),
                            )
                        for u in range(4):
                            hb = 4 * g + u
                            j = hb % 32
                            nc.vector.copy_predicated(
                                attn_a[0:32, :],
                                identu8[0:32, j:j + 1].broadcast_to((32, 128)),
                                ps_a[0:32, u * 128:(u + 1) * 128],
                            )
                # attnT block = attn_a^T + vnewT block (f16 for o_proj)
                ps_aT = pstr.tile([128, 32], F32, tag="tr")
                nc.tensor.transpose(ps_aT[:], attn_a[:], id32)
                nc.vector.tensor_add(
                    attnT[:, pa:pa + 32], ps_aT[:], vnewT[:, pa:pa + 32]
                )

            # ---------------- phase C: o_part = attn^T @ Wo_c, chunked RS ----------
            o_rss = []
            for r, (n0, n1) in enumerate(ROUNDS):
                nn = n1 - n0
                ps_os = [
                    psbank.tile([8, 512], F32, tag="bank", name=f"ps_o{r}_{i}")
                    for i in range(nn)
                ]
                for hp in range(8):
                    wo_t = wop.tile([128, 2 * 6 * 512], F16, tag="wo")
                    nc.sync.dma_start(out=wo_t[:, 0:2 * nn * 512], in_=wos[r][hp])
                    for i2 in range(2):
                        h = 2 * hp + i2
                        for i in range(nn):
                            nc.tensor.matmul(
                                ps_os[i][:8, :],
                                attnT[:, h * 8:(h + 1) * 8],
                                wo_t[:, (i2 * nn + i) * 512:(i2 * nn + i + 1) * 512],
                                start=(h == 0), stop=(h == 15),
                            )
                ostage = stg.tile([8, 6 * 512], F32, tag="ostage")
                for i in range(nn):
                    nc.vector.tensor_copy(
                        ostage[:, i * 512:(i + 1) * 512], ps_os[i][:8, :]
                    )
                o_bounce = dram.tile([B, nn * 512], F32, tag=f"ob{r}")
                nc.sync.dma_start(out=o_bounce[:], in_=ostage[:, 0:nn * 512])
                o_rs = dram.tile([1, nn * 512], F32, tag=f"ors{r}")
                nc.gpsimd.collective_compute(
                    "ReduceScatter",
                    mybir.AluOpType.add,
                    replica_groups=rg,
                    ins=[o_bounce.opt()],
                    outs=[o_rs.opt()],
                )
                o_rss.append((o_rs, n0, n1))

            for o_rs, n0, n1 in o_rss:
                nc.sync.dma_start(out=o[0:1, n0 * 512:n1 * 512], in_=o_rs[:])

    nc.compile()
    return nc


_NC_CACHE = None


def _get_nc():
    global _NC_CACHE
    if _NC_CACHE is None:
        _NC_CACHE = build_nc()
    return _NC_CACHE


def make_in_maps(x, k_cache, v_cache, Wq_down, Wq_up, Wkv_down, Wv_up, Wo):
    f16 = np.float16
    x2 = np.asarray(x, dtype=np.float32).reshape(B, HID).T  # [7168, 8]
    xt_t = np.ascontiguousarray(
        x2.reshape(56, 128, B).transpose(1, 0, 2).reshape(128, 56 * B).astype(BF)
    )
    Wq_down = np.asarray(Wq_down, dtype=np.float32)
    Wq_up = np.asarray(Wq_up, dtype=np.float32)
    Wkv_down = np.asarray(Wkv_down, dtype=np.float32)
    Wv_up = np.asarray(Wv_up, dtype=np.float32)
    Wo = np.asarray(Wo, dtype=np.float32)
    k_cache = np.asarray(k_cache, dtype=np.float32)
    v_cache = np.asarray(v_cache, dtype=np.float32)

    in_maps = []
    for c in range(NC_):
        hs = slice(c * HP, (c + 1) * HP)
        wqd_c = (
            Wq_down[:, c * QLP:(c + 1) * QLP]
            .reshape(2, 28, 128, QLP).transpose(0, 2, 1, 3)
            .reshape(2, 128, 28 * QLP).astype(BF)
        )
        wkvd_c = (
            Wkv_down[c * KVRP:(c + 1) * KVRP, :]
            .reshape(7, 128, KVL).transpose(1, 0, 2).reshape(128, 7 * KVL)
            .astype(BF)
        )
        xkv_c = np.ascontiguousarray(
            x2.reshape(56, 128, B)[7 * c:7 * c + 7]
            .transpose(1, 0, 2).reshape(128, 7 * B).astype(BF)
        )
        wq_shard = Wq_up[:, c * NH:(c + 1) * NH]
        # pad rows to 256 per rank (rows r*256+192..255 zero) so the K=64
        # chunk sits at partitions 0..64 of its own x-column
        wq_pad = np.zeros((2048, NH), np.float32)
        for r in range(8):
            wq_pad[r * 256:r * 256 + QLP] = wq_shard[r * QLP:(r + 1) * QLP]
        wq_c = np.stack([
            wq_pad[:, n * 512:(n + 1) * 512]
            .reshape(16, 128, 512).transpose(1, 0, 2).reshape(128, 16 * 512)
            for n in range(4)
        ]).astype(BF)
        wvup_c = (
            Wv_up[:, c * NH:(c + 1) * NH]
            .reshape(2, 2, 128, NH).transpose(0, 2, 1, 3).reshape(2, 128, 2 * NH)
            .astype(BF)
        )
        kt_c = (
            k_cache[:, hs]
            .transpose(1, 0, 3, 2)          # (16, 8, 128, 512) [h, b, d, k]
            .reshape(32, 4, 128, 512)       # [g, t, d, k]
            .transpose(0, 2, 1, 3)          # [g, d, t, k]
            .reshape(16, 2, 128, 2048)      # [s, g2, d, tk]
            .transpose(0, 2, 1, 3)
            .reshape(16, 128, 4096)
            .astype(BF)
        )
        v_c = (
            v_cache[:, hs]
            .transpose(1, 0, 2, 3)          # (16, 8, 512, 128) [h, b, l, d]
            .reshape(32, 4, 4, 128, 128)    # [g, t, c, k, d]
            .transpose(0, 3, 2, 1, 4)       # [g, k, c, t, d]
            .reshape(16, 2, 128, 2048)
            .transpose(0, 2, 1, 3)
            .reshape(16, 128, 4096)
            .astype(BF)
        )
        wo_shard = Wo[c * NH:(c + 1) * NH, :]  # [2048, 7168]
        wo_cs = []
        for (n0, n1) in ROUNDS:
            nn = n1 - n0
            wo_r = (
                wo_shard[:, n0 * 512:n1 * 512]
                .reshape(8, 2, 128, nn * 512).transpose(0, 2, 1, 3)
                .reshape(8, 128, 2 * nn * 512).astype(f16)
            )
            wo_cs.append(wo_r)
        in_maps.append(
            {
                "xt": xt_t,
                "xkv": xkv_c,
                "wqd": np.ascontiguousarray(wqd_c),
                "wkvd": np.ascontiguousarray(wkvd_c),
                "wq": np.ascontiguousarray(wq_c),
                "wvup": np.ascontiguousarray(wvup_c),
                "kt": np.ascontiguousarray(kt_c),
                "v": np.ascontiguousarray(v_c),
                "wo0": np.ascontiguousarray(wo_cs[0]),
                "wo1": np.ascontiguousarray(wo_cs[1]),
                "wo2": np.ascontiguousarray(wo_cs[2]),
            }
        )
    return in_maps


def kernel(x, k_cache, v_cache, Wq_down, Wq_up, Wkv_down, Wk_up, Wv_up, Wo, **_):
    in_maps = make_in_maps(
        np.asarray(x), np.asarray(k_cache), np.asarray(v_cache),
        np.asarray(Wq_down), np.asarray(Wq_up),
        np.asarray(Wkv_down), np.asarray(Wv_up), np.asarray(Wo),
    )
    nc = _get_nc()
    res = bass_utils.run_bass_kernel_spmd(nc, in_maps, core_ids=list(range(NC_)))
    out = np.stack([res.results[b]["o"] for b in range(B)], axis=0)  # (8, 1, 7168)
    return np.ascontiguousarray(out, dtype=np.float32)
